# revision 1
# baseline (speedup 1.0000x reference)
"""Trainium2 Bass kernel for BaseRelationNetwork forward pass.

Reference computation (per batch row b):
    pairs (i<j) of C=16 channels, P=120 pairs
    h1 = relu(concat(x_i, x_j) @ W1 + b1)      # W1 [2F, H]
    h2 = relu(h1 @ W2 + b2)
    out = mean_p(h2 @ W3 + b3)                 # [B, H]

Algebraic restructuring used here:
  1. W1 splits into W1a (top F rows, applied to x_i) and W1b (bottom F rows,
     applied to x_j). ya = x @ W1a and yb = x @ W1b are computed once per
     channel (C matmuls) instead of per pair (P matmuls): 7.5x less PE work.
     h1[p=(i,j)] = relu(ya[i] + yb[j] + b1) is a cheap DVE gather-add.
  2. mean over pairs commutes with the affine layer 3:
     out = (mean_p h2) @ W3 + b3. Layer 3 runs on the pair-mean only.

Sharding: data-parallel over batch. 512 rows / 8 cores = 64 rows per core.
Weights replicated. Host pre-transposes x to feature-major layout with
token = chunk*256 + c*16 + b (batch split into 4 chunks of 16) so the
pipeline (layer-1 matmul -> pair-add -> layer-2 -> accumulate) runs as
overlapping chunks; the 1/P mean scale is folded into W3 and the biases
are packed into one [128, 6] tile on the host.

Everything is bfloat16 except the PSUM accumulations, biases and W3: bf16
halves DMA bytes and keeps the PE at 1 cycle/row; the DVE runs 2x
(tensor_tensor) / 4x (tensor_scalar) on packed 16-bit SBUF operands, so
the pair-add runs at 2 elem/cycle/lane and the fused b1+relu
(tensor_scalar add+max) at 4 elem/cycle/lane. Measured rel err ~9e-4.

The emission is software-pipelined: PE stream is [L1(h) | L2(h-1) |
flush(h-2)], so layer-2 of a chunk runs only after layer-1 of the next
chunk, by which time its DVE chain (drain, pair-add, relu) has finished
and the PE never stalls on it. x/W1 use partition-major DRAM layouts
(contiguous KB-scale runs per partition, measured ~1.5x faster DMA) and
stream over the sync + gpsimd queues in first-use order.
"""

import contextlib
import sys

if "/opt/trn_rl_repo" not in sys.path:
    sys.path.insert(0, "/opt/trn_rl_repo")

import numpy as np
import ml_dtypes

import concourse.bass as bass
import concourse.mybir as mybir
import concourse.tile as tile
from concourse import bacc
from concourse.bass_utils import run_bass_kernel_spmd

# Problem shape (hardcoded per contract).
B, C, F, H = 512, 16, 1024, 256
N_CORES = 8
BL = B // N_CORES          # 64 local batch rows per core
P = C * (C - 1) // 2       # 120 pairs
NH = 4                     # batch chunks per core (chunked pipeline)
BH = BL // NH              # 16 rows per chunk
TOK = BL * C               # 1024 tokens per core
HTOK = BH * C              # 256 tokens per chunk, token = chunk*256 + c*16 + b
F32 = mybir.dt.float32
F32R = mybir.dt.float32r
BF16 = mybir.dt.bfloat16

KT1 = F // 128             # 8 k-tiles for layer-1 contraction
KQ = 4                     # k-tiles per merged x DMA
PPG = 30                   # pairs per stage-C sub-group
GW = PPG * BH              # stage-C sub-group width: 480 columns
NG = P // PPG              # 4 stage-C sub-groups per chunk
NSP = NG // 2              # 2 double-width (960-col) stage-C groups per chunk
FSW = 160                  # flush moving-slice width (10 pairs x 16 b): the
                           # identity matmuls accumulate 12 slices into a
                           # [128, 160] psum so the DVE reduce reads only 160

# pair enumeration: for i in 0..C-2, j in i+1..C-1, p consecutive
PAIR_BASE = [0] * C
for _i in range(1, C):
    PAIR_BASE[_i] = PAIR_BASE[_i - 1] + (C - 1 - (_i - 1))

AF = mybir.ActivationFunctionType
ALU = mybir.AluOpType


def build_module(loop_iters: int = 1, dma_in_loop: bool = True, l1_sweep: bool = False, warmup: int = 24, debug: bool = True):
    nc = bacc.Bacc("TRN2", target_bir_lowering=False, debug=debug)

    # partition-major DRAM layouts: per-partition data is contiguous, so
    # DMA descriptors cover KB-scale runs (measured ~1.5x faster than the
    # row-major [F, TOK] layout on hardware)
    xt_d = nc.dram_tensor("xt", [128, NH, KT1, HTOK], BF16, kind="ExternalInput")
    w1_d = nc.dram_tensor("w1", [128, 2 * KT1, H], BF16, kind="ExternalInput")
    w2_d = nc.dram_tensor("w2", [H, H], BF16, kind="ExternalInput")
    w3_d = nc.dram_tensor("w3", [H, H], F32, kind="ExternalInput")
    bp_d = nc.dram_tensor("bias_pack", [128, 6], F32, kind="ExternalInput")
    id_d = nc.dram_tensor("ident", [128, 128], BF16, kind="ExternalInput")
    out_d = nc.dram_tensor("outT", [H, BL], F32, kind="ExternalOutput")

    with tile.TileContext(nc) as tc:
        with (
            tc.tile_pool(name="xpool", bufs=1) as xpool,
            tc.tile_pool(name="wpool", bufs=1) as wpool,
            tc.tile_pool(name="ypool", bufs=1) as ypool,
            tc.tile_pool(name="hpool", bufs=1) as hpool,
            tc.tile_pool(name="spool", bufs=1) as spool,
            tc.tile_pool(name="psA", bufs=2, space="PSUM") as psA_pool,
            tc.tile_pool(name="psC", bufs=6, space="PSUM") as psC_pool,
        ):
            # big tiles (bufs=1 pools: same buffers every loop iteration)
            xts = xpool.tile([128, NH, KT1, HTOK], BF16, tag="xts", name="xts")
            w1big = wpool.tile([128, 2 * KT1, H], BF16, tag="w1big", name="w1big")
            w2t = wpool.tile([128, 2, H], BF16, tag="w2t", name="w2t")
            w3t = wpool.tile([128, 2, H], F32, tag="w3t", name="w3t")
            bp = wpool.tile([128, 6], F32, tag="bp", name="bp")
            idt = wpool.tile([128, 128], BF16, tag="idt", name="idt")
            # y_all free layout: [m(4), chunk(NH), c(C), b(BH)]
            y_all = ypool.tile([128, 4, TOK], BF16, tag="y_all", name="y_all")
            # h1 free layout: [t(2), chunk(NH), p(P), b(BH)]
            h1all = hpool.tile(
                [128, 2, NH * P * BH], BF16, tag="h1all", name="h1all"
            )
            h2sb = [
                [
                    spool.tile(
                        [128, GW * NG], BF16,
                        tag=f"h2_{m}_{par}", name=f"h2_{m}_{par}",
                    )
                    for par in range(2)
                ]
                for m in range(2)
            ]
            m2 = [
                spool.tile([128, BL], F32, tag=f"m2_{m}", name=f"m2_{m}")
                for m in range(2)
            ]
            osb = spool.tile([128, 2, BL], F32, tag="osb", name="osb")

            def bias(nm, t):
                idx = {"b1": 0, "b2": 2, "b3": 4}[nm] + t
                return bp[:, idx : idx + 1]

            w1v = w1_d
            xtv = xt_d

            def hs(half):
                return slice(half * HTOK, (half + 1) * HTOK)

            def emit_dmas():
                # ---- DMA: two queues issue in parallel, ordered by first
                # use. sync (HWDGE): x chunk-0 halves + W1b + late weights;
                # gpsimd (SWDGE): W1a + x chunks 1-3. bias_pack is off the
                # critical path (first use ~8us in).
                # Layer 1 can start once x[ch0,k0-3] + W1[k0-3] land ----
                ka, kb = slice(0, KQ), slice(KQ, KT1)
                x0sl = slice(0, 2) if l1_sweep else slice(0, 1)
                nc.sync.dma_start(
                    out=xts[:, x0sl, ka, :], in_=xtv[:, x0sl, ka, :]
                )
                nc.gpsimd.dma_start(out=w1big[:, ka, :], in_=w1v[:, ka, :])
                nc.sync.dma_start(
                    out=w1big[:, KT1 + 0 : KT1 + KQ, :],
                    in_=w1v[:, KT1 + 0 : KT1 + KQ, :],
                )
                nc.sync.dma_start(
                    out=xts[:, x0sl, kb, :], in_=xtv[:, x0sl, kb, :]
                )
                nc.gpsimd.dma_start(out=w1big[:, kb, :], in_=w1v[:, kb, :])
                nc.sync.dma_start(
                    out=w1big[:, KT1 + KQ : 2 * KT1, :],
                    in_=w1v[:, KT1 + KQ : 2 * KT1, :],
                )
                nc.sync.dma_start(out=bp[:], in_=bp_d[:])
                nc.sync.dma_start(
                    out=w2t[:], in_=w2_d.rearrange("(k p) h -> p k h", p=128)
                )
                nc.sync.dma_start(out=idt[:], in_=id_d[:])
                for ch in range(2 if l1_sweep else 1, NH):
                    nc.gpsimd.dma_start(
                        out=xts[:, ch, :, :], in_=xtv[:, ch, :, :]
                    )
                nc.sync.dma_start(
                    out=w3t[:], in_=w3_d.rearrange("(k p) h -> p k h", p=128)
                )

            if not dma_in_loop:
                emit_dmas()

            loop_cm = (
                tc.For_i(0, loop_iters, 1)
                if loop_iters > 1
                else contextlib.nullcontext()
            )
            with loop_cm:
                if dma_in_loop:
                    emit_dmas()

                def flush_acc_m(ph, ppar, m):
                    # sum 12 FSW-wide slices (4 su-blocks x 3 sub-slices)
                    # on PE: identity pass-through matmuls accumulating in
                    # a narrow PSUM so the DVE reduce reads only FSW elems.
                    nsl = (GW * NG) // FSW
                    psr = psC_pool.tile(
                        [128, FSW], F32, tag="psC", name=f"psR_{ph}_{m}"
                    )
                    for su in range(nsl):
                        nc.tensor.matmul(
                            psr[:],
                            idt[:],
                            h2sb[m][ppar][:, su * FSW : (su + 1) * FSW],
                            start=(su == 0),
                            stop=(su == nsl - 1),
                        )
                    # then reduce over p only: [128, b, p] view
                    v = psr.rearrange("q (pp b) -> q pp b", b=BH).transpose(
                        [0, 2, 1]
                    )
                    nc.vector.tensor_reduce(
                        m2[m][:, ph * BH : (ph + 1) * BH],
                        v,
                        mybir.AxisListType.X,
                        ALU.add,
                    )

                # PE warm-up while DMAs stream: dummy matmuls on a memset
                # tile (no DMA dependency) bridge the PE until the first
                # real operands land, keeping the HAM clock gate open
                # full-array dummies: a [128,128] stationary and [128,240]
                # output keep the whole PE array active so the HAM activity
                # monitor registers the warm-up (a 1-partition output may
                # count as near-idle and never open the clock gate)
                wsrc = spool.tile([128, 128], BF16, tag="wsrc", name="wsrc")
                if warmup:
                    nc.vector.memset(wsrc[:], 0.0)
                    warm = psA_pool.tile(
                        [128, HTOK], F32, tag="psA", name="warm"
                    )
                for _ in range(warmup):
                    nc.tensor.matmul(
                        warm[:, :240],
                        wsrc[:],
                        wsrc[:, 0:1].broadcast_to([128, 240]),
                        start=True,
                        stop=True,
                    )

                # pairs use only ya[i<15] and yb[j>0]: the ya matmuls skip
                # channel 15's tokens and the yb matmuls skip channel 0's
                # (first/last BH token columns), 6% less layer-1 work
                ATOK = HTOK - BH

                def atk(m):
                    return slice(0, ATOK) if m < 2 else slice(BH, HTOK)

                def stage_A(half):
                    # layer-1 matmuls. Chunk 0 runs all four m tiles in one
                    # k-sweep (borrowing two idle psC bufs) so it consumes
                    # the just-arriving k-quads at full width; later chunks
                    # use two m-pair passes so only two psA PSUM tiles are
                    # live at a time (the psC pool needs the banks by then).
                    if half == 0:
                        passes = ((0, 2, 1, 3),)
                    else:
                        passes = ((0, 2), (1, 3))
                    for mp in passes:
                        psA = {}
                        for m in mp:
                            if half == 0 and m >= 1 and m != 2:
                                t = psC_pool.tile(
                                    [128, 512], F32, tag="psC",
                                    name=f"psA_{half}_{m}",
                                )
                                psA[m] = t[:, :ATOK]
                            else:
                                psA[m] = psA_pool.tile(
                                    [128, ATOK], F32, tag="psA",
                                    name=f"psA_{half}_{m}",
                                )[:]
                        for k in range(KT1):
                            for m in mp:
                                w_half, ht = divmod(m, 2)
                                nc.tensor.matmul(
                                    psA[m],
                                    w1big[
                                        :, w_half * KT1 + k,
                                        ht * 128 : (ht + 1) * 128,
                                    ],
                                    xts[:, half, k, atk(m)],
                                    start=(k == 0),
                                    stop=(k == KT1 - 1),
                                )
                        # b1 is folded into the fused relu pass: plain copies
                        for m in mp:
                            off = half * HTOK + (0 if m < 2 else BH)
                            dst = y_all[:, m, off : off + ATOK]
                            if m < 2:
                                nc.vector.tensor_scalar_add(dst, psA[m], 0.0)
                            else:
                                nc.scalar.copy(dst, psA[m])

                def stage_A_sweep(sw):
                    # layer-1 for a chunk pair: 480-col moving operands
                    # halve the matmul and LDWEIGHTS count vs per-chunk
                    for mp in ((0, 2), (1, 3)):
                        psA = {
                            m: psA_pool.tile(
                                [128, 2, ATOK], F32, tag="psA",
                                name=f"psAs_{sw}_{m}",
                            )
                            for m in mp
                        }
                        for k in range(KT1):
                            for m in mp:
                                w_half, ht = divmod(m, 2)
                                nc.tensor.matmul(
                                    psA[m][:],
                                    w1big[
                                        :, w_half * KT1 + k,
                                        ht * 128 : (ht + 1) * 128,
                                    ],
                                    xts[:, 2 * sw : 2 * sw + 2, k, atk(m)],
                                    start=(k == 0),
                                    stop=(k == KT1 - 1),
                                )
                        for m in mp:
                            for j in range(2):
                                half = sw * 2 + j
                                off = half * HTOK + (0 if m < 2 else BH)
                                dst = y_all[:, m, off : off + ATOK]
                                if m < 2:
                                    nc.vector.tensor_scalar_add(
                                        dst, psA[m][:, j, :], 0.0
                                    )
                                else:
                                    nc.scalar.copy(dst, psA[m][:, j, :])

                def stage_B(half):
                    # pair-add on DVE (bf16 2x mode), then fused b1-add +
                    # relu on DVE (bf16 4x tensor_scalar)
                    hbase = half * P * BH
                    y5 = y_all.rearrange("p m (hh c b) -> p m hh c b", hh=NH, b=BH)
                    h5 = h1all.rearrange("p t (hh pp b) -> p t hh pp b", hh=NH, b=BH)
                    for i in range(C - 1):
                        nj = C - 1 - i
                        p0 = PAIR_BASE[i]
                        in0 = y5[:, 0:2, half, i : i + 1, :].broadcast_to(
                            [128, 2, nj, BH]
                        )
                        in1 = y5[:, 2:4, half, i + 1 :, :]
                        outap = h5[:, :, half, p0 : p0 + nj, :]
                        nc.vector.tensor_add(outap, in0, in1)
                    for t in range(2):
                        sl = h1all[:, t, hbase : hbase + P * BH]
                        nc.vector.tensor_scalar(
                            sl, sl, bias("b1", t), 0.0, ALU.add, ALU.max
                        )

                def stage_C_mm(half, m):
                    # layer-2 matmuls for one (chunk, m) pair; k-outer so
                    # the stationary w2t tile changes only twice
                    hbase = half * P * BH
                    pst = [
                        psC_pool.tile(
                            [128, 512], F32, tag="psC",
                            name=f"psC_{half}_{m}_{s}",
                        )
                        for s in range(NG)
                    ]
                    for k in range(2):
                        for s in range(NG):
                            nc.tensor.matmul(
                                pst[s][:, :GW],
                                w2t[:, k, m * 128 : (m + 1) * 128],
                                h1all[
                                    :, k, hbase + s * GW : hbase + (s + 1) * GW
                                ],
                                start=(k == 0),
                                stop=(k == 1),
                            )
                    return pst

                def stage_C_drain(half, m, pst, s, engine):
                    par = half % 2
                    h2t = h2sb[m][par][:, s * GW : (s + 1) * GW]
                    if engine == "act":
                        nc.scalar.activation(
                            h2t, pst[s][:, :GW], AF.Relu, bias=bias("b2", m)
                        )
                    else:
                        # relu(+b2) on DVE: (psum + b2) max 0 in one pass
                        nc.vector.tensor_scalar(
                            h2t, pst[s][:, :GW], bias("b2", m), 0.0,
                            ALU.add, ALU.max,
                        )

                def stage_C(half, split_drains=False):
                    # split_drains: ACT/DVE alternation halves the psC pool
                    # recycle latency. Only safe late in the schedule, when
                    # the DVE queue is past all pair-add/relu work.
                    for m in range(2):
                        pst = stage_C_mm(half, m)
                        if split_drains:
                            for s in (0, 2, 1, 3):
                                stage_C_drain(
                                    half, m, pst, s,
                                    "act" if s % 2 == 0 else "dve",
                                )
                        else:
                            for s in range(NG):
                                stage_C_drain(half, m, pst, s, "act")

                def flush_chunk(half):
                    par = half % 2
                    for m in range(2):
                        flush_acc_m(half, par, m)

                # ---- software-pipelined emission. PE stream:
                #   L1(h) | C(h-1) | F(h-2): layer-2 of chunk h only runs
                # after layer-1 of chunk h+1, by which time the DVE chain
                # (drain, pair-add, relu) of chunk h has long finished ----
                if l1_sweep:
                    stage_A_sweep(0)
                    stage_B(0)
                    stage_B(1)
                    stage_A_sweep(1)
                    stage_B(2)
                    stage_B(3)
                    stage_C(0)
                    stage_C(1)
                    flush_chunk(0)
                    stage_C(2)
                    flush_chunk(1)
                else:
                    for half in range(NH):
                        stage_A(half)
                        stage_B(half)
                        if half >= 1:
                            stage_C(half - 1)
                        if half >= 2:
                            flush_chunk(half - 2)

                # ---- tail: last chunk's layer-2, drains split ACT/DVE and
                # flushes interleaved so the serial end chain is short ----
                lh = NH - 1
                lpar = lh % 2
                pst0 = stage_C_mm(lh, 0)
                for s in (0, 2, 1, 3):
                    stage_C_drain(lh, 0, pst0, s, "act" if s % 2 == 0 else "dve")
                flush_chunk(lh - 1)
                pst1 = stage_C_mm(lh, 1)
                for s in (0, 2, 1, 3):
                    stage_C_drain(lh, 1, pst1, s, "act" if s % 2 == 0 else "dve")
                flush_acc_m(lh, lpar, 0)
                flush_acc_m(lh, lpar, 1)

                # ---- stage E: outT = (m2 @ W3scaled) + b3 (bias on DVE).
                # k-outer: both k=0 matmuls run while the DVE finishes the
                # last flush reduce (m2[1]), shortening the serial tail ----
                outv = out_d.rearrange("(m p) b -> p m b", p=128)
                psE = {}
                for k in range(2):
                    for mo in range(2):
                        if k == 0:
                            psE[mo] = psA_pool.tile(
                                [128, HTOK], F32, tag="psA", name=f"psE_{mo}"
                            )
                        nc.tensor.matmul(
                            psE[mo][:, :BL],
                            w3t[:, k, mo * 128 : (mo + 1) * 128],
                            m2[k][:],
                            start=(k == 0),
                            stop=(k == 1),
                        )
                for mo in range(2):
                    nc.vector.tensor_scalar_add(
                        osb[:, mo, :], psE[mo][:, :BL], bias("b3", mo)
                    )
                nc.sync.dma_start(out=outv[:], in_=osb[:])

    nc.compile()
    return nc


_NC_CACHE = None


def _get_module():
    global _NC_CACHE
    if _NC_CACHE is None:
        _NC_CACHE = build_module()
    return _NC_CACHE


def make_in_maps(x, W1, b1, W2, b2, W3, b3):
    # W1 in partition-major layout [128, 2*KT1, H]:
    # element (p, k, h) = W1[k*128 + p, h]
    W1 = np.ascontiguousarray(
        np.asarray(W1, dtype=np.float32)
        .astype(ml_dtypes.bfloat16)
        .reshape(2 * KT1, 128, H)
        .transpose(1, 0, 2)
    )
    w3p = np.ascontiguousarray(W3, dtype=np.float32) / np.float32(P)
    b1 = np.asarray(b1, dtype=np.float32)
    b2 = np.asarray(b2, dtype=np.float32)
    b3 = np.asarray(b3, dtype=np.float32)
    bias_pack = np.stack(
        [b1[:128], b1[128:], b2[:128], b2[128:], b3[:128], b3[128:]], axis=1
    )
    bias_pack = np.ascontiguousarray(bias_pack, dtype=np.float32)
    w2bf = np.ascontiguousarray(
        np.asarray(W2, dtype=np.float32).astype(ml_dtypes.bfloat16)
    )
    idbf = np.eye(128, dtype=ml_dtypes.bfloat16)
    in_maps = []
    for i in range(N_CORES):
        xs = x[i * BL : (i + 1) * BL]  # [BL, C, F]
        # token within a chunk = c*BH + b; feature-major then partition-major:
        # xT4[p, ch, k, t] = x[ch*BH + b, c, k*128 + p]
        halves = [
            xs[h * BH : (h + 1) * BH].transpose(1, 0, 2).reshape(HTOK, F)
            for h in range(NH)
        ]
        xT = np.concatenate(halves, axis=0).T.astype(ml_dtypes.bfloat16)  # [F, TOK]
        xT4 = np.ascontiguousarray(
            xT.reshape(KT1, 128, NH, HTOK).transpose(1, 2, 0, 3)
        )
        in_maps.append(
            {
                "xt": xT4,
                "w1": W1,
                "w2": w2bf,
                "w3": np.ascontiguousarray(w3p, dtype=np.float32),
                "bias_pack": bias_pack,
                "ident": idbf,
            }
        )
    return in_maps


def kernel(x, W1, b1, W2, b2, W3, b3):
    nc = _get_module()
    in_maps = make_in_maps(
        np.asarray(x, dtype=np.float32),
        np.asarray(W1),
        np.asarray(b1),
        np.asarray(W2),
        np.asarray(b2),
        np.asarray(W3),
        np.asarray(b3),
    )
    res = run_bass_kernel_spmd(nc, in_maps, list(range(N_CORES)))
    out = np.empty((B, H), dtype=np.float32)
    for i in range(N_CORES):
        out[i * BL : (i + 1) * BL] = res.results[i]["outT"].T
    return out



# revision 43
# speedup vs baseline: 1.8363x; 1.8363x over previous
"""Trainium2 Bass kernel for BaseRelationNetwork forward pass.

Reference computation (per batch row b):
    pairs (i<j) of C=16 channels, P=120 pairs
    h1 = relu(concat(x_i, x_j) @ W1 + b1)      # W1 [2F, H]
    h2 = relu(h1 @ W2 + b2)
    out = mean_p(h2 @ W3 + b3)                 # [B, H]

Algebraic restructuring (as the bf16 baseline):
  1. W1 splits into W1a/W1b; ya = x @ W1a, yb = x @ W1b computed once per
     channel; h1[p=(i,j)] = relu(ya[i] + yb[j] + b1) via DVE pair-adds.
  2. mean over pairs commutes with affine layer 3; layer 3 runs on the
     pair-mean only.

This version runs layers 1 and 2 in fp8 e4m3 with DoubleRow perf mode:
the PE contracts two 128-deep k-tiles per pass (2 MACs/cell/cycle),
halving matmul time vs bf16. Scales keep fp8 operands out of the
subnormal range: W1 is pre-scaled by S1=256 and W2 by S2=32 on the
host; the descales are folded into existing elementwise passes (relu
pass divides by S1; W3 absorbs 1/(S2*P)), so no extra instructions are
needed. h1 is stored fp8 at natural scale; h2 and the pair-sum flush
(identity matmuls) stay bf16. Measured rel err ~1.1e-2 (gate 2e-2).

Elementwise work is split between DVE and ACT only: DVE keeps the
pair-adds (tensor_tensor 2x), the t1 relu+fp8-casts in-chain and late
drains; ACT takes the t0 relus, psA copies and most layer-2 psum
drains. GPSIMD does DMA descriptor work ONLY — its tensor ops measure
~10x the cost model on HW (one [128,1920] tensor_scalar ~ 24us), and
it cannot access PSUM at all (BIR verifier rule).

Sharding: data-parallel over batch, 64 rows/core, weights replicated.
Host pre-transposes x to a DoubleRow-friendly partition-major layout
[p, sweep, kpair, ko, chl, token] (token = c*16 + b within a 16-row
chunk); biases pre-scaled and packed into one [128, 6] tile.
"""

import contextlib
import sys

if "/opt/trn_rl_repo" not in sys.path:
    sys.path.insert(0, "/opt/trn_rl_repo")

import numpy as np
import ml_dtypes

import concourse.bass as bass
import concourse.mybir as mybir
import concourse.tile as tile
from concourse import bacc
from concourse.bass_utils import run_bass_kernel_spmd

# Problem shape (hardcoded per contract).
B, C, F, H = 512, 16, 1024, 256
N_CORES = 8
BL = B // N_CORES          # 64 local batch rows per core
P = C * (C - 1) // 2       # 120 pairs
NH = 4                     # batch chunks per core (chunked pipeline)
BH = BL // NH              # 16 rows per chunk
TOK = BL * C               # 1024 tokens per core
HTOK = BH * C              # 256 tokens per chunk, token = c*16 + b
F32 = mybir.dt.float32
BF16 = mybir.dt.bfloat16
FP8 = mybir.dt.float8e4

S1 = 256.0                 # host pre-scale on W1 (fp8 subnormal avoidance)
S2 = 32.0                  # host pre-scale on W2

KT1 = F // 128             # 8 k-tiles for layer-1 contraction
KQ = KT1 // 2              # 4 DoubleRow k-pairs
PPG = 30                   # pairs per stage-C sub-group
GW = PPG * BH              # stage-C sub-group width: 480 columns
NG = P // PPG              # 4 stage-C sub-groups per chunk
FSW = 160                  # flush moving-slice width (10 pairs x 16 b)

# pair enumeration: for i in 0..C-2, j in i+1..C-1, p consecutive
PAIR_BASE = [0] * C
for _i in range(1, C):
    PAIR_BASE[_i] = PAIR_BASE[_i - 1] + (C - 1 - (_i - 1))

AF = mybir.ActivationFunctionType
ALU = mybir.AluOpType
DR = mybir.MatmulPerfMode.DoubleRow


DEFAULT_SCHED = {
    # engines for the 8 psA->y copies (PSUM src: dve/act only)
    "a0_copies": ("act", "dve", "dve", "act"),
    "a1_copies": ("act", "act", "act", "act"),
    # relu+cast engines per chunk: (t0, t1).
    # NOTE: GPSIMD tensor ops measure ~10x the cost model on HW (the Q7
    # software path) — never assign compute to it, DMA only.
    "relu": {0: ("act", "dve"), 1: ("act", "dve"),
             2: ("act", "dve"), 3: ("act", "dve")},
    # prompt drains per chunk 0..2 (PSUM src: dve/act only)
    "prompt": {ch: ((0, "act"), (2, "act")) for ch in (0, 1, 2)},
    # deferred drains per chunk 0..2
    "deferred": {ch: ((1, "act"), (3, "dve")) for ch in (0, 1, 2)},
    # last-chunk drains, both m
    "c3_drains": ((0, "dve"), (2, "act"), (1, "dve"), (3, "act")),
    "preload_act": True,
}


def build_module(loop_iters: int = 1, dma_in_loop: bool = True, warmup: int = 20, debug: bool = True, sched: dict | None = None):
    sched = {**DEFAULT_SCHED, **(sched or {})}
    nc = bacc.Bacc("TRN2", target_bir_lowering=False, debug=debug)

    # partition-major DRAM layouts (contiguous per-partition runs)
    xt_d = nc.dram_tensor("xt", [128, 2, KQ, 2, 2, HTOK], FP8, kind="ExternalInput")
    w1_d = nc.dram_tensor("w1", [128, 2 * KQ, 2, H], FP8, kind="ExternalInput")
    w2_d = nc.dram_tensor("w2", [128, 2, H], FP8, kind="ExternalInput")
    w3_d = nc.dram_tensor("w3", [H, H], F32, kind="ExternalInput")
    bp_d = nc.dram_tensor("bias_pack", [128, 6], F32, kind="ExternalInput")
    id_d = nc.dram_tensor("ident", [128, 128], BF16, kind="ExternalInput")
    out_d = nc.dram_tensor("outT", [H, BL], F32, kind="ExternalOutput")

    with tile.TileContext(nc) as tc:
        with (
            tc.tile_pool(name="xpool", bufs=1) as xpool,
            tc.tile_pool(name="wpool", bufs=1) as wpool,
            tc.tile_pool(name="ypool", bufs=1) as ypool,
            tc.tile_pool(name="hpool", bufs=1) as hpool,
            tc.tile_pool(name="spool", bufs=1) as spool,
            tc.tile_pool(name="psA", bufs=3, space="PSUM") as psA_pool,
            tc.tile_pool(name="psC", bufs=5, space="PSUM") as psC_pool,
        ):
            # big tiles (bufs=1 pools: same buffers every loop iteration)
            xts = xpool.tile([128, 2, KQ, 2, 2, HTOK], FP8, tag="xts", name="xts")
            w1big = wpool.tile([128, 2 * KQ, 2, H], FP8, tag="w1big", name="w1big")
            w2t = wpool.tile([128, 2, H], FP8, tag="w2t", name="w2t")
            w3t = wpool.tile([128, 2, H], F32, tag="w3t", name="w3t")
            bp = wpool.tile([128, 6], F32, tag="bp", name="bp")
            idt = wpool.tile([128, 128], BF16, tag="idt", name="idt")
            # y_all free layout: [m(4), chunk(NH), c(C), b(BH)]
            y_all = ypool.tile([128, 4, TOK], BF16, tag="y_all", name="y_all")
            # pre-relu pair-sums, bf16, ping-pong by chunk parity
            h1tmp = [
                ypool.tile([128, 2, P * BH], BF16, tag=f"h1t_{par}", name=f"h1t_{par}")
                for par in range(2)
            ]
            # h1 free layout: [t(2), chunk(NH), p(P), b(BH)], fp8 natural scale
            h1all = hpool.tile([128, 2, NH * P * BH], FP8, tag="h1all", name="h1all")
            h2sb = [
                [
                    spool.tile(
                        [128, GW * NG], BF16,
                        tag=f"h2_{m}_{ch}", name=f"h2_{m}_{ch}",
                    )
                    for ch in range(NH)
                ]
                for m in range(2)
            ]
            m2 = [
                spool.tile([128, BL], F32, tag=f"m2_{m}", name=f"m2_{m}")
                for m in range(2)
            ]
            osb = spool.tile([128, 2, BL], F32, tag="osb", name="osb")

            def bias(nm, t):
                idx = {"b1": 0, "b2": 2, "b3": 4}[nm] + t
                return bp[:, idx : idx + 1]

            def emit_dmas():
                # sync (HWDGE): bias first (copies fold b1), x sweep-0 per
                # k-pair (layer-1 starts after ~190KB), then W1b + late
                # weights. gpsimd (SWDGE): W1a (needed by the first m-pass)
                # + x sweep-1 — the Pool engine does DMA work only.
                nc.sync.dma_start(out=bp[:], in_=bp_d[:])
                nc.gpsimd.dma_start(out=w1big[:, 0:KQ], in_=w1_d[:, 0:KQ])
                nc.sync.dma_start(out=xts[:, 0], in_=xt_d[:, 0])
                nc.sync.dma_start(out=w1big[:, KQ : 2 * KQ], in_=w1_d[:, KQ : 2 * KQ])
                nc.gpsimd.dma_start(out=xts[:, 1], in_=xt_d[:, 1])
                nc.sync.dma_start(out=w2t[:], in_=w2_d[:])
                nc.sync.dma_start(out=idt[:], in_=id_d[:])
                nc.sync.dma_start(out=w3t[:], in_=w3_d.rearrange("(k p) h -> p k h", p=128))

            if not dma_in_loop:
                emit_dmas()

            loop_cm = (
                tc.For_i(0, loop_iters, 1)
                if loop_iters > 1
                else contextlib.nullcontext()
            )
            with loop_cm:
                if dma_in_loop:
                    emit_dmas()

                # PE warm-up while DMAs stream (HAM clock gate)
                wsrc = spool.tile([128, 128], BF16, tag="wsrc", name="wsrc")
                if warmup:
                    nc.vector.memset(wsrc[:], 0.0)
                    warm = psA_pool.tile([128, 256], F32, tag="psA", name="warm")
                if sched["preload_act"]:
                    # trigger the ACT function-table load during the DMA
                    # wait, not in front of the first real activation
                    nc.scalar.activation(wsrc[:, 0:1], wsrc[:, 0:1], AF.Relu)
                for _ in range(warmup):
                    nc.tensor.matmul(
                        warm[:, :240],
                        wsrc[:],
                        wsrc[:, 0:1].broadcast_to([128, 240]),
                        start=True,
                        stop=True,
                    )

                def stage_A(sw, copy_engines):
                    # layer-1 DoubleRow matmuls for a chunk pair.
                    # m = (w_half, t): 0 = ya t0, 1 = ya t1, 2 = yb t0, 3 = yb t1
                    # (ya passes first: W1a arrives before W1b)
                    for mp in ((0, 1), (2, 3)):
                        psA = {
                            m: psA_pool.tile(
                                [128, 2, HTOK], F32, tag="psA",
                                name=f"psA_{sw}_{m}",
                            )
                            for m in mp
                        }
                        for kp in range(KQ):
                            for m in mp:
                                w_half, ht = divmod(m, 2)
                                nc.tensor.matmul(
                                    psA[m].rearrange("q c t -> q (c t)"),
                                    w1big[:, w_half * KQ + kp, :, ht * 128 : (ht + 1) * 128],
                                    xts[:, sw, kp],
                                    start=(kp == 0),
                                    stop=(kp == KQ - 1),
                                    perf_mode=DR,
                                )
                        # drain psA -> y (bf16). ya gets +S1*b1 folded in; the
                        # 1/S1 descale is folded into the relu+cast pass.
                        for m in mp:
                            dst = y_all[:, m, sw * 2 * HTOK : (sw + 1) * 2 * HTOK]
                            src = psA[m].rearrange("q c t -> q (c t)")
                            eng = copy_engines[m]
                            if m < 2:
                                b1s = bias("b1", m)
                                if eng == "act":
                                    nc.scalar.activation(dst, src, AF.Identity, bias=b1s)
                                elif eng == "gpsimd":
                                    nc.gpsimd.tensor_scalar_add(dst, src, b1s)
                                else:
                                    nc.vector.tensor_scalar_add(dst, src, b1s)
                            else:
                                if eng == "act":
                                    nc.scalar.copy(dst, src)
                                elif eng == "gpsimd":
                                    nc.gpsimd.tensor_copy(dst, src)
                                else:
                                    nc.vector.tensor_scalar_add(dst, src, 0.0)

                def stage_B(half, relu_engines):
                    # pair-add on DVE (bf16 2x), into bf16 h1tmp; then fused
                    # relu + 1/S1 descale + fp8 cast into h1all.
                    par = half % 2
                    y5 = y_all.rearrange("p m (hh c b) -> p m hh c b", hh=NH, b=BH)
                    h5t = h1tmp[par].rearrange("p t (pp b) -> p t pp b", b=BH)
                    for i in range(C - 1):
                        nj = C - 1 - i
                        p0 = PAIR_BASE[i]
                        in0 = y5[:, 0:2, half, i : i + 1, :].broadcast_to(
                            [128, 2, nj, BH]
                        )
                        in1 = y5[:, 2:4, half, i + 1 :, :]
                        outap = h5t[:, :, p0 : p0 + nj, :]
                        nc.vector.tensor_add(outap, in0, in1)
                    hbase = half * P * BH
                    hp = (
                        tc.high_priority()
                        if sched.get("hp_relu")
                        else contextlib.nullcontext()
                    )
                    with hp:
                        stage_B_relus(half, par, hbase, relu_engines)

                def stage_B_relus(half, par, hbase, relu_engines):
                    for t in range(2):
                        src = h1tmp[par][:, t, :]
                        dst = h1all[:, t, hbase : hbase + P * BH]
                        eng = relu_engines[t]
                        if eng == "act":
                            nc.scalar.activation(dst, src, AF.Relu, scale=1.0 / S1)
                        elif eng == "gpsimd":
                            nc.gpsimd.tensor_scalar(
                                dst, src, 0.0, 1.0 / S1, ALU.max, ALU.mult
                            )
                        else:
                            nc.vector.tensor_scalar(
                                dst, src, 0.0, 1.0 / S1, ALU.max, ALU.mult
                            )

                def stage_C_mm(half, m):
                    # layer-2 DoubleRow matmuls for one (chunk, m) pair
                    hbase = half * P * BH
                    pst = [
                        psC_pool.tile(
                            [128, 512], F32, tag="psC",
                            name=f"psC_{half}_{m}_{s}",
                        )
                        for s in range(NG)
                    ]
                    for s in range(NG):
                        nc.tensor.matmul(
                            pst[s][:, :GW],
                            w2t[:, :, m * 128 : (m + 1) * 128],
                            h1all[:, :, hbase + s * GW : hbase + (s + 1) * GW],
                            start=True,
                            stop=True,
                            perf_mode=DR,
                        )
                    return pst

                def stage_C_drain(half, m, pst, s, engine):
                    # h2 = relu(z + S2*b2), stored fp8 at scale S2
                    h2t = h2sb[m][half][:, s * GW : (s + 1) * GW]
                    src = pst[s][:, :GW]
                    if engine == "act":
                        nc.scalar.activation(
                            h2t, src, AF.Relu, bias=bias("b2", m)
                        )
                    elif engine == "gpsimd":
                        nc.gpsimd.tensor_scalar(
                            h2t, src, bias("b2", m), 0.0, ALU.add, ALU.max
                        )
                    else:
                        nc.vector.tensor_scalar(
                            h2t, src, bias("b2", m), 0.0, ALU.add, ALU.max
                        )

                def flush_mm(ph, m):
                    # sum 12 FSW-wide slices on PE via bf16 identity
                    # matmuls; DVE p-reduce emitted later.
                    nsl = (GW * NG) // FSW          # 12
                    psr = psA_pool.tile(
                        [128, FSW], F32, tag="psA", name=f"psR_{ph}_{m}"
                    )
                    for su in range(nsl):
                        nc.tensor.matmul(
                            psr[:],
                            idt[:],
                            h2sb[m][ph][:, su * FSW : (su + 1) * FSW],
                            start=(su == 0),
                            stop=(su == nsl - 1),
                        )
                    return psr

                def flush_reduce(ph, m, psr):
                    v = psr.rearrange("q (pp b) -> q pp b", b=BH).transpose(
                        [0, 2, 1]
                    )
                    nc.vector.tensor_reduce(
                        m2[m][:, ph * BH : (ph + 1) * BH],
                        v,
                        mybir.AxisListType.X,
                        ALU.add,
                    )

                # ---- software-pipelined emission, in (approximate) time
                # order so every per-engine FIFO sees its ops in the order
                # they become runnable ----
                stage_A(0, copy_engines=sched["a0_copies"])
                stage_B(0, relu_engines=sched["relu"][0])
                stage_B(1, relu_engines=sched["relu"][1])
                stage_A(1, copy_engines=sched["a1_copies"])

                pst = {}
                for ch in (0, 1):
                    for m in range(2):
                        pst[(ch, m)] = stage_C_mm(ch, m)
                        for s, eng in sched["prompt"][ch]:
                            stage_C_drain(ch, m, pst[(ch, m)], s, eng)
                    stage_B(2 + ch, relu_engines=sched["relu"][2 + ch])

                for m in range(2):
                    pst[(2, m)] = stage_C_mm(2, m)
                    for s, eng in sched["prompt"][2]:
                        stage_C_drain(2, m, pst[(2, m)], s, eng)

                # deferred drains: DVE is free once the B-chain ends
                for ch in (0, 1, 2):
                    for m in range(2):
                        for s, eng in sched["deferred"][ch]:
                            stage_C_drain(ch, m, pst[(ch, m)], s, eng)

                psr = {}
                for m in range(2):
                    psr[(0, m)] = flush_mm(0, m)
                for m in range(2):
                    flush_reduce(0, m, psr[(0, m)])
                    psr[(1, m)] = flush_mm(1, m)

                # ---- tail: last chunk, drains spread over all three
                # elementwise engines, flushes interleaved ----
                lh = NH - 1
                pst0 = stage_C_mm(lh, 0)
                for s, eng in sched["c3_drains"]:
                    stage_C_drain(lh, 0, pst0, s, eng)
                for m in range(2):
                    flush_reduce(1, m, psr[(1, m)])
                    psr[(2, m)] = flush_mm(2, m)
                pst1 = stage_C_mm(lh, 1)
                for s, eng in sched["c3_drains"]:
                    stage_C_drain(lh, 1, pst1, s, eng)
                for m in range(2):
                    flush_reduce(2, m, psr[(2, m)])
                    psr[(3, m)] = flush_mm(3, m)
                for m in range(2):
                    flush_reduce(3, m, psr[(3, m)])

                # ---- stage E: outT = (m2 @ W3scaled) + b3 (bias on DVE) ----
                outv = out_d.rearrange("(m p) b -> p m b", p=128)
                psE = {}
                for k in range(2):
                    for mo in range(2):
                        if k == 0:
                            psE[mo] = psA_pool.tile(
                                [128, 256], F32, tag="psA", name=f"psE_{mo}"
                            )
                        nc.tensor.matmul(
                            psE[mo][:, :BL],
                            w3t[:, k, mo * 128 : (mo + 1) * 128],
                            m2[k][:],
                            start=(k == 0),
                            stop=(k == 1),
                        )
                for mo in range(2):
                    nc.vector.tensor_scalar_add(
                        osb[:, mo, :], psE[mo][:, :BL], bias("b3", mo)
                    )
                nc.sync.dma_start(out=outv[:], in_=osb[:])

    nc.compile()
    return nc


_NC_CACHE = None


def _get_module():
    global _NC_CACHE
    if _NC_CACHE is None:
        _NC_CACHE = build_module()
    return _NC_CACHE


def _to_fp8(a):
    return np.clip(np.asarray(a, dtype=np.float32), -240.0, 240.0).astype(
        ml_dtypes.float8_e4m3
    )


def make_in_maps(x, W1, b1, W2, b2, W3, b3):
    x = np.asarray(x, dtype=np.float32)
    # W1 layout [p, (wh kq), ko, H]: element = W1[wh*F + (2*kp+ko)*128 + p, h] * S1
    W1q = _to_fp8(np.asarray(W1, dtype=np.float32) * S1)
    W1q = np.ascontiguousarray(
        W1q.reshape(2, KQ, 2, 128, H).transpose(3, 0, 1, 2, 4).reshape(128, 2 * KQ, 2, H)
    )
    # W2 layout [p, ko, H]: element = W2[ko*128 + p, h] * S2
    W2q = _to_fp8(np.asarray(W2, dtype=np.float32) * S2)
    W2q = np.ascontiguousarray(W2q.reshape(2, 128, H).transpose(1, 0, 2))
    w3p = np.ascontiguousarray(W3, dtype=np.float32) / np.float32(P * S2)
    b1 = np.asarray(b1, dtype=np.float32) * np.float32(S1)
    b2 = np.asarray(b2, dtype=np.float32) * np.float32(S2)
    b3 = np.asarray(b3, dtype=np.float32)
    bias_pack = np.stack(
        [b1[:128], b1[128:], b2[:128], b2[128:], b3[:128], b3[128:]], axis=1
    )
    bias_pack = np.ascontiguousarray(bias_pack, dtype=np.float32)
    id2 = np.eye(128, dtype=ml_dtypes.bfloat16)
    in_maps = []
    for i in range(N_CORES):
        xs = x[i * BL : (i + 1) * BL]  # [BL, C, F]
        # token within a chunk = c*BH + b
        halves = [
            xs[h * BH : (h + 1) * BH].transpose(1, 0, 2).reshape(HTOK, F)
            for h in range(NH)
        ]
        xT = np.concatenate(halves, axis=0).T  # [F, TOK] f32
        # [p, sw, kp, ko, chl, t]: xT[(2*kp+ko)*128 + p, (2*sw+chl)*HTOK + t]
        xT6 = xT.reshape(KQ, 2, 128, 2, 2, HTOK).transpose(2, 3, 0, 1, 4, 5)
        in_maps.append(
            {
                "xt": np.ascontiguousarray(_to_fp8(xT6)),
                "w1": W1q,
                "w2": W2q,
                "w3": w3p,
                "bias_pack": bias_pack,
                "ident": id2,
            }
        )
    return in_maps


def kernel(x, W1, b1, W2, b2, W3, b3):
    nc = _get_module()
    in_maps = make_in_maps(x, W1, b1, W2, b2, W3, b3)
    res = run_bass_kernel_spmd(nc, in_maps, list(range(N_CORES)))
    out = np.empty((B, H), dtype=np.float32)
    for i in range(N_CORES):
        out[i * BL : (i + 1) * BL] = res.results[i]["outT"].T
    return out
